# revision 37
# baseline (speedup 1.0000x reference)
"""MCR loss kernel for Trainium2 (8 NeuronCores).

Strategy:
  - Shard batch T=16 -> 2 timesteps per core (data parallel, no collectives).
  - Host converts inputs to bf16: halves HBM traffic (the roofline term) and
    enables the DVE 2x packed mode for the pooling adds.
  - Per core, 6 plane-groups (2 timesteps x 3 maps).  Each group's 32x192x192
    plane stack is one contiguous 2.25MB DMA into [128=(c,quarter), 9216].
  - 8x8 avg-pool (as sum; 1/64 folded into conv weights) via a 6-op
    tensor_tensor pairwise tree on the vector engine (2x mode on bf16).
  - Reflect-pad + dy-replication built by small SBUF->SBUF DMAs on the
    scalar-engine HWDGE ring; 3x3 conv as 3 PE matmuls with K=(dy,ic)=96;
    LeakyReLU(0.2) natively on the scalar engine (Lrelu, PSUM->SBUF).
  - Gram G_t = V_t V_t^T via PE transpose + bf16 matmul (f32 PSUM accum).
  - Host: matrix determinant lemma
        logdet(I_576 + a V^T V) = logdet(I_96 + a V V^T)
    so only the [2,96,96] Grams leave the device; float64 Cholesky logdets
    finish the scalar loss.
"""

import numpy as np
import ml_dtypes

_STATE = {}

# -------- fixed problem geometry (hardcoded per harness contract) --------
B, CCH, H, W = 16, 32, 192, 192
NCORES = 8
TPC = B // NCORES          # timesteps per core = 2
NG = TPC * 3               # plane groups per core
OUT = 24                   # pooled spatial size
PIX = OUT * OUT            # 576
M = 96                     # feature rows (3 maps x 32 channels)
ALPHA_E = 6.0              # 576 / (96 * eps)
ALPHA_C = 18.0             # 576 / (32 * eps)

DEBUG_TAPS = False


def _build_nc():
    import concourse.bass as bass
    import concourse.tile as tile
    from concourse import bacc, mybir

    BF = mybir.dt.bfloat16
    F8 = mybir.dt.float8e4
    F32 = mybir.dt.float32
    ACT = mybir.ActivationFunctionType
    OP = mybir.AluOpType

    nc = bacc.Bacc(
        "TRN2", target_bir_lowering=False, debug=False, num_devices=NCORES
    )

    x = nc.declare_dram_parameter("x", [NG, CCH, H, W], F8, isOutput=False)
    wt = nc.declare_dram_parameter("wt", [96, 288], BF, isOutput=False)
    ident = nc.declare_dram_parameter("ident", [96, 96], BF, isOutput=False)
    # sel[:, q*32+c] = e_{c*4+q}: column-permuted identity; sel[:, q*32:...]
    # as matmul lhsT gathers partition (c,q) -> output row c
    sel = nc.declare_dram_parameter("sel", [128, 128], BF, isOutput=False)
    g_out = nc.declare_dram_parameter("g_out", [TPC, M, M], F32, isOutput=True)
    if DEBUG_TAPS:
        pooled_out = nc.declare_dram_parameter(
            "pooled_out", [NG * 32, PIX], F32, isOutput=True
        )
        v_out = nc.declare_dram_parameter("v_out", [96, TPC * PIX], F32, isOutput=True)

    with tile.TileContext(nc) as tc:
        with (
            tc.tile_pool(name="persist", bufs=1) as persist,
            tc.tile_pool(name="slab", bufs=1) as slab_pool,
            tc.tile_pool(name="red", bufs=3) as red_pool,
            tc.tile_pool(name="xrep", bufs=2) as xrep_pool,
            tc.tile_pool(name="vt", bufs=2) as vt_pool,
            tc.tile_pool(name="psc", bufs=2, space="PSUM") as psc_pool,
            tc.tile_pool(name="pst", bufs=2, space="PSUM") as pst_pool,
            tc.tile_pool(name="psg", bufs=1, space="PSUM") as psg_pool,
            tc.tile_pool(name="psx", bufs=1, space="PSUM") as psx_pool,
        ):
            wt_sb = persist.tile([96, 288], BF, tag="wt")
            nc.sync.dma_start(out=wt_sb[:], in_=wt.ap())
            id_sb = persist.tile([96, 96], BF, tag="id")
            nc.sync.dma_start(out=id_sb[:], in_=ident.ap())
            sel_sb = persist.tile([128, 128], BF, tag="sel")
            nc.sync.dma_start(out=sel_sb[:], in_=sel.ap())
            v_sb = persist.tile([96, TPC * PIX], BF, tag="v")
            g_sb = persist.tile([96, TPC * 96], F32, tag="g")

            # ---- prefetch all 6 slabs up front: each is one contiguous
            # 2.25MB DMA, partition=(c, quarter).  Dispatching them before
            # any small DMA keeps the 8 shared HWDGE completion-semaphore
            # lanes fresh — interleaving would chain slab loads behind tiny
            # sbuf-to-sbuf DMAs from two groups earlier.
            slabs = []
            for g in range(NG):
                slab = slab_pool.tile([128, 9216], BF, tag=f"slab{g}")
                # SWDGE cast-DMA: reads fp8 from HBM, writes bf16 to SBUF.
                # Split-slab DMAs for finer pipelining: slab 0 in quarters
                # (the very first L1 gates the whole pipeline), the rest in
                # halves.
                xg = x.ap()[g].rearrange("c (q h) w -> (c q) (h w)", q=4)
                nsplit = 4 if g == 0 else (2 if g == NG - 1 else 1)
                step = 9216 // nsplit
                for s in range(nsplit):
                    nc.gpsimd.dma_start(
                        out=slab[:, s * step : (s + 1) * step],
                        in_=xg[:, s * step : (s + 1) * step],
                    )
                slabs.append(slab)

            gram_ps = {}
            for g in range(NG):
                t, m = divmod(g, 3)
                slab = slabs[g]

                # ---- pooling: pairwise tensor_tensor tree (bf16 2x mode) ----
                # per partition: 48 rows x 192 cols = (48h, 24x, 8w).
                # L1 split into row-blocks matching the split-slab DMAs.
                sv = slab[:].rearrange("p (h x w) -> p h x w", h=48, x=24, w=8)
                t1 = red_pool.tile([128, 4608], BF, tag="t1")
                t1v = t1[:].rearrange("p (h x w) -> p h x w", h=48, x=24, w=4)
                nsplit = 4 if g == 0 else (2 if g == NG - 1 else 1)
                rstep = 48 // nsplit
                for s in range(nsplit):
                    r0, r1 = s * rstep, (s + 1) * rstep
                    nc.vector.tensor_tensor(
                        out=t1v[:, r0:r1],
                        in0=sv[:, r0:r1, :, 0:4],
                        in1=sv[:, r0:r1, :, 4:8],
                        op=OP.add,
                    )
                t2 = red_pool.tile([128, 2304], BF, tag="t2")
                t2v = t2[:].rearrange("p (h x w) -> p h x w", h=48, x=24, w=2)
                nc.vector.tensor_tensor(
                    out=t2v, in0=t1v[:, :, :, 0:2], in1=t1v[:, :, :, 2:4], op=OP.add
                )
                # h-direction 8:1 before the final w-pair: (6y, 8r, 48xw)
                t2r = t2[:].rearrange("p (y r s) -> p y r s", y=6, r=8, s=48)
                t3 = red_pool.tile([128, 1152], BF, tag="t3")
                t3v = t3[:].rearrange("p (y r s) -> p y r s", y=6, r=4, s=48)
                nc.vector.tensor_tensor(
                    out=t3v, in0=t2r[:, :, 0:4, :], in1=t2r[:, :, 4:8, :], op=OP.add
                )
                t4 = red_pool.tile([128, 576], BF, tag="t4")
                t4v = t4[:].rearrange("p (y r s) -> p y r s", y=6, r=2, s=48)
                nc.vector.tensor_tensor(
                    out=t4v, in0=t3v[:, :, 0:2, :], in1=t3v[:, :, 2:4, :], op=OP.add
                )
                t5 = red_pool.tile([128, 288], BF, tag="t5")
                t5v = t5[:].rearrange("p (y r s) -> p y r s", y=6, r=1, s=48)
                nc.vector.tensor_tensor(
                    out=t5v, in0=t4v[:, :, 0:1, :], in1=t4v[:, :, 1:2, :], op=OP.add
                )
                # final w-pair written straight into the x-padded 26-wide row
                # layout: pooled [128=(c,q), (6y, 26x)], cols 1..24 are data
                t5w = t5[:].rearrange("p (y x w) -> p y x w", y=6, x=24, w=2)
                pooled = red_pool.tile([128, 6 * 26], BF, tag="pooled")
                pv26 = pooled[:].rearrange("p (y x) -> p y x", y=6, x=26)
                pv0 = pv26[:, :, 1:25].rearrange("p y (x w) -> p y x w", w=1)
                nc.vector.tensor_tensor(
                    out=pv0, in0=t5w[:, :, :, 0:1], in1=t5w[:, :, :, 1:2], op=OP.add
                )
                # x reflect pads, lane-local (cols 0,25 <- cols 2,23).
                # Mid-stream on ACT (DVE is the pacing engine there); the
                # last group's on the then-idle DVE for minimum tail latency.
                if g == NG - 1:
                    nc.vector.tensor_copy(pv26[:, :, 0:1], pv26[:, :, 2:3])
                    nc.vector.tensor_copy(pv26[:, :, 25:26], pv26[:, :, 23:24])
                else:
                    nc.scalar.copy(out=pv26[:, :, 0:1], in_=pv26[:, :, 2:3])
                    nc.scalar.copy(out=pv26[:, :, 25:26], in_=pv26[:, :, 23:24])

                # ---- build xrep [96=(dy,c), 24y, 26x] on the TENSOR engine.
                # Selector matmuls (lhsT = column-permuted identity slice)
                # remap partitions (c,q) -> (dy,c) and place dy-shifted,
                # reflect-padded row windows — no DMA involved, so the conv
                # never queues behind the HBM slab stream.  Two PSUM tiles
                # hold element halves 0:312 / 312:624 of the 624-el rows.
                psA = psx_pool.tile([96, 312], F32, tag="xpsA")
                psB = psx_pool.tile([96, 312], F32, tag="xpsB")
                # (q, dy-block, tile, dest el0:el1, src el0:el1); dy block d
                # writes row r from source row r + d - 1 (reflect at edges)
                pieces = [
                    (0, 0, psA, 0, 26, 26, 52),      # d0 row0 <- reflect row1
                    (0, 0, psA, 26, 182, 0, 156),    # d0 rows1-6 <- q0 rows0-5
                    (0, 1, psA, 0, 156, 0, 156),     # d1 rows0-5
                    (0, 2, psA, 0, 130, 26, 156),    # d2 rows0-4 <- q0 rows1-5
                    (1, 0, psA, 182, 312, 0, 130),   # d0 rows7-11
                    (1, 0, psB, 0, 26, 130, 156),    # d0 row12
                    (1, 1, psA, 156, 312, 0, 156),   # d1 rows6-11
                    (1, 2, psA, 130, 286, 0, 156),   # d2 rows5-10
                    (2, 0, psB, 26, 182, 0, 156),    # d0 rows13-18
                    (2, 1, psB, 0, 156, 0, 156),     # d1 rows12-17
                    (2, 2, psA, 286, 312, 0, 26),    # d2 row11
                    (2, 2, psB, 0, 130, 26, 156),    # d2 rows12-16
                    (3, 0, psB, 182, 312, 0, 130),   # d0 rows19-23
                    (3, 1, psB, 156, 312, 0, 156),   # d1 rows18-23
                    (3, 2, psB, 130, 286, 0, 156),   # d2 rows17-22
                    (3, 2, psB, 286, 312, 104, 130), # d2 row23 <- reflect row22
                ]
                for q, dblk, psX, e0, e1, s0, s1 in pieces:
                    nc.tensor.matmul(
                        psX[dblk * 32 : (dblk + 1) * 32, e0:e1],
                        sel_sb[:, q * 32 : (q + 1) * 32],
                        pooled[:, s0:s1],
                        start=True,
                        stop=True,
                    )
                xrep = xrep_pool.tile([96, 24 * 26], BF, tag="xrep")
                # last group: DVE is idle by now and is faster PSUM->SBUF;
                # mid-stream: keep these off the busy DVE
                ceng = nc.vector.tensor_copy if g == NG - 1 else (
                    lambda out, in_: nc.scalar.copy(out=out, in_=in_)
                )
                ceng(xrep[:, 0:312], psA[:])
                ceng(xrep[:, 312:624], psB[:])
                xr3 = xrep[:].rearrange("p (y x) -> p y x", y=OUT, x=26)

                if DEBUG_TAPS:
                    nc.gpsimd.dma_start(
                        out=pooled_out.ap()[g * 32 : (g + 1) * 32],
                        in_=xr3[32:64, :, 1:25],
                    )

                # ---- conv: 2 halves x 3 dx matmuls, K=(dy,ic)=96 ------------
                for half in range(2):
                    pc = psc_pool.tile([32, 288], F32, tag="convps")
                    for dx in range(3):
                        nc.tensor.matmul(
                            pc[:],
                            wt_sb[:, (m * 3 + dx) * 32 : (m * 3 + dx + 1) * 32],
                            xr3[:, 12 * half : 12 * half + 12, dx : dx + 24],
                            start=(dx == 0),
                            stop=(dx == 2),
                        )
                    # LeakyReLU(0.2) on the scalar engine, PSUM -> SBUF bf16
                    nc.scalar.activation(
                        out=v_sb[
                            m * 32 : (m + 1) * 32,
                            t * PIX + half * 288 : t * PIX + (half + 1) * 288,
                        ],
                        in_=pc[:],
                        func=ACT.Prelu,
                        alpha=0.2,
                    )

                # ---- Gram chunks: t0's five chunks spread over groups
                # 3/4/5 keep the tensor engine from idling >3.4us
                # mid-stream (HAM re-throttle), so tail matmuls run warm.
                def gram_chunks(gt, c5s, last_group):
                    gp = gram_ps[gt]
                    for c5 in c5s:
                        sz = 128 if c5 < 4 else 64
                        vsl = v_sb[
                            :, gt * PIX + c5 * 128 : gt * PIX + c5 * 128 + sz
                        ]
                        pt = pst_pool.tile([128, 96], BF, tag="vtps")
                        nc.tensor.transpose(pt[:sz, :], vsl, id_sb[:])
                        vtt = vt_pool.tile([128, 96], BF, tag="vt")
                        if last_group:
                            nc.vector.tensor_copy(vtt[:sz, :], pt[:sz, :])
                        else:
                            nc.scalar.copy(out=vtt[:sz, :], in_=pt[:sz, :])
                        nc.tensor.matmul(
                            gp[:], vtt[:sz, :], vtt[:sz, :],
                            start=(c5 == 0), stop=(c5 == 4),
                        )
                    if c5s[-1] == 4:
                        nc.scalar.copy(
                            out=g_sb[:, gt * 96 : (gt + 1) * 96], in_=gp[:]
                        )

                if g == 3:
                    gram0 = psg_pool.tile([96, 96], F32, tag="gram0")
                    gram_ps[0] = gram0
                    gram_chunks(0, [0, 1], False)
                elif g == 4:
                    gram_chunks(0, [2, 3], False)
                elif g == 5:
                    gram_chunks(0, [4], False)
                    gram1 = psg_pool.tile([96, 96], F32, tag="gram1")
                    gram_ps[1] = gram1
                    gram_chunks(1, [0, 1, 2, 3, 4], True)
            # one store for both timesteps' Grams
            nc.sync.dma_start(
                out=g_out.ap().rearrange("t i k -> i t k"), in_=g_sb[:]
            )

    nc.finalize()
    return nc


def _get_nc():
    if "nc" not in _STATE:
        _STATE["nc"] = _build_nc()
    return _STATE["nc"]


def _prep_weights(W1, W2, W3):
    # wt[(dy,ic), (m,dx,oc)] = W_m[oc, ic, dy, dx] / 64   (pool-mean folded in)
    w = np.stack([np.asarray(Wi, np.float64) for Wi in (W1, W2, W3)])
    wt = w.transpose(3, 2, 0, 4, 1).reshape(96, 288) / 64.0
    return wt.astype(ml_dtypes.bfloat16)


def _host_loss(G):
    G = np.asarray(G, np.float64)  # [16, 96, 96]
    T = G.shape[0]
    I96 = np.eye(M)
    Me = I96[None] + ALPHA_E * G
    ld_e = 2.0 * np.log(
        np.diagonal(np.linalg.cholesky(Me), axis1=-2, axis2=-1)
    ).sum()
    blocks = np.stack(
        [G[:, 32 * c : 32 * (c + 1), 32 * c : 32 * (c + 1)] for c in range(3)]
    )  # [3, T, 32, 32]
    Mc = np.eye(32)[None, None] + ALPHA_C * blocks
    ld_c = 2.0 * np.log(
        np.diagonal(np.linalg.cholesky(Mc), axis1=-2, axis2=-1)
    ).sum()
    loss_expd = ld_e / (2.0 * T)
    loss_comp = (32.0 / M) * ld_c / (2.0 * T)
    return np.float32(loss_expd - loss_comp)


def run_device(inputs, **kw):
    """Run the bass kernel; returns (G [16,96,96], BassKernelResults)."""
    from concourse.bass_utils import run_bass_kernel_spmd

    nc = _get_nc()
    wt = _prep_weights(inputs["W1"], inputs["W2"], inputs["W3"])
    ident = np.eye(96, dtype=ml_dtypes.bfloat16)
    # sel[:, q*32+c] = e_{c*4+q}
    perm = np.arange(128).reshape(4, 32).T.reshape(-1) * 0
    perm = np.array([(i % 32) * 4 + (i // 32) for i in range(128)])
    selm = np.eye(128)[:, perm].astype(ml_dtypes.bfloat16)
    ms = np.asarray(inputs["ms_fea"], np.float32)
    pan = np.asarray(inputs["pan_fea"], np.float32)
    alf = np.asarray(inputs["all_fea"], np.float32)
    in_maps = []
    for i in range(NCORES):
        sl = slice(TPC * i, TPC * (i + 1))
        # x[t*3+m] = (ms,pan,alf)[m][t]
        xs = np.stack([ms[sl], pan[sl], alf[sl]], axis=1).reshape(
            NG, CCH, H, W
        )
        in_maps.append(
            {"x": xs.astype(ml_dtypes.float8_e4m3fn), "wt": wt, "ident": ident,
             "sel": selm}
        )
    res = run_bass_kernel_spmd(nc, in_maps, core_ids=list(range(NCORES)), **kw)
    G = np.concatenate([np.asarray(r["g_out"]) for r in res.results], axis=0)
    return G, res


def kernel(**inputs):
    G, _ = run_device(inputs)
    return _host_loss(G)


# revision 38
# speedup vs baseline: 1.0228x; 1.0228x over previous
"""MCR loss kernel for Trainium2 (8 NeuronCores).

Strategy:
  - Shard batch T=16 -> 2 timesteps per core (data parallel, no collectives).
  - Host converts inputs to bf16: halves HBM traffic (the roofline term) and
    enables the DVE 2x packed mode for the pooling adds.
  - Per core, 6 plane-groups (2 timesteps x 3 maps).  Each group's 32x192x192
    plane stack is one contiguous 2.25MB DMA into [128=(c,quarter), 9216].
  - 8x8 avg-pool (as sum; 1/64 folded into conv weights) via a 6-op
    tensor_tensor pairwise tree on the vector engine (2x mode on bf16).
  - Reflect-pad + dy-replication built by small SBUF->SBUF DMAs on the
    scalar-engine HWDGE ring; 3x3 conv as 3 PE matmuls with K=(dy,ic)=96;
    LeakyReLU(0.2) natively on the scalar engine (Lrelu, PSUM->SBUF).
  - Gram G_t = V_t V_t^T via PE transpose + bf16 matmul (f32 PSUM accum).
  - Host: matrix determinant lemma
        logdet(I_576 + a V^T V) = logdet(I_96 + a V V^T)
    so only the [2,96,96] Grams leave the device; float64 Cholesky logdets
    finish the scalar loss.
"""

import numpy as np
import ml_dtypes

_STATE = {}

# -------- fixed problem geometry (hardcoded per harness contract) --------
B, CCH, H, W = 16, 32, 192, 192
NCORES = 8
TPC = B // NCORES          # timesteps per core = 2
NG = TPC * 3               # plane groups per core
OUT = 24                   # pooled spatial size
PIX = OUT * OUT            # 576
M = 96                     # feature rows (3 maps x 32 channels)
ALPHA_E = 6.0              # 576 / (96 * eps)
ALPHA_C = 18.0             # 576 / (32 * eps)

DEBUG_TAPS = False


def _build_nc():
    import concourse.bass as bass
    import concourse.tile as tile
    from concourse import bacc, mybir

    BF = mybir.dt.bfloat16
    F8 = mybir.dt.float8e4
    F32 = mybir.dt.float32
    ACT = mybir.ActivationFunctionType
    OP = mybir.AluOpType

    nc = bacc.Bacc(
        "TRN2", target_bir_lowering=False, debug=False, num_devices=NCORES
    )

    x = nc.declare_dram_parameter("x", [NG, CCH, H, W], F8, isOutput=False)
    wt = nc.declare_dram_parameter("wt", [96, 288], BF, isOutput=False)
    ident = nc.declare_dram_parameter("ident", [96, 96], BF, isOutput=False)
    # sel[:, q*32+c] = e_{c*4+q}: column-permuted identity; sel[:, q*32:...]
    # as matmul lhsT gathers partition (c,q) -> output row c
    sel = nc.declare_dram_parameter("sel", [128, 128], BF, isOutput=False)
    g_out = nc.declare_dram_parameter("g_out", [TPC, M, M], F32, isOutput=True)
    if DEBUG_TAPS:
        pooled_out = nc.declare_dram_parameter(
            "pooled_out", [NG * 32, PIX], F32, isOutput=True
        )
        v_out = nc.declare_dram_parameter("v_out", [96, TPC * PIX], F32, isOutput=True)

    with tile.TileContext(nc) as tc:
        with (
            tc.tile_pool(name="persist", bufs=1) as persist,
            tc.tile_pool(name="slab", bufs=1) as slab_pool,
            tc.tile_pool(name="red", bufs=3) as red_pool,
            tc.tile_pool(name="xrep", bufs=2) as xrep_pool,
            tc.tile_pool(name="vt", bufs=2) as vt_pool,
            tc.tile_pool(name="psc", bufs=2, space="PSUM") as psc_pool,
            tc.tile_pool(name="pst", bufs=2, space="PSUM") as pst_pool,
            tc.tile_pool(name="psg", bufs=1, space="PSUM") as psg_pool,
            tc.tile_pool(name="psx", bufs=1, space="PSUM") as psx_pool,
        ):
            wt_sb = persist.tile([96, 288], BF, tag="wt")
            nc.sync.dma_start(out=wt_sb[:], in_=wt.ap())
            id_sb = persist.tile([96, 96], BF, tag="id")
            nc.sync.dma_start(out=id_sb[:], in_=ident.ap())
            sel_sb = persist.tile([128, 128], BF, tag="sel")
            nc.sync.dma_start(out=sel_sb[:], in_=sel.ap())
            v_sb = persist.tile([96, TPC * PIX], BF, tag="v")
            g_sb = persist.tile([96, TPC * 96], F32, tag="g")

            # ---- prefetch all 6 slabs up front: each is one contiguous
            # 2.25MB DMA, partition=(c, quarter).  Dispatching them before
            # any small DMA keeps the 8 shared HWDGE completion-semaphore
            # lanes fresh — interleaving would chain slab loads behind tiny
            # sbuf-to-sbuf DMAs from two groups earlier.
            slabs = []
            for g in range(NG):
                slab = slab_pool.tile([128, 9216], BF, tag=f"slab{g}")
                # SWDGE cast-DMA: reads fp8 from HBM, writes bf16 to SBUF.
                # Split-slab DMAs for finer pipelining: slab 0 in quarters
                # (the very first L1 gates the whole pipeline), the rest in
                # halves.
                xg = x.ap()[g].rearrange("c (q h) w -> (c q) (h w)", q=4)
                nsplit = 4 if g == 0 else 2
                step = 9216 // nsplit
                for s in range(nsplit):
                    nc.gpsimd.dma_start(
                        out=slab[:, s * step : (s + 1) * step],
                        in_=xg[:, s * step : (s + 1) * step],
                    )
                slabs.append(slab)

            gram_ps = {}
            for g in range(NG):
                t, m = divmod(g, 3)
                slab = slabs[g]

                # ---- pooling: pairwise tensor_tensor tree (bf16 2x mode) ----
                # per partition: 48 rows x 192 cols = (48h, 24x, 8w).
                # L1 split into row-blocks matching the split-slab DMAs.
                sv = slab[:].rearrange("p (h x w) -> p h x w", h=48, x=24, w=8)
                t1 = red_pool.tile([128, 4608], BF, tag="t1")
                t1v = t1[:].rearrange("p (h x w) -> p h x w", h=48, x=24, w=4)
                nsplit = 4 if g == 0 else 2
                rstep = 48 // nsplit
                for s in range(nsplit):
                    r0, r1 = s * rstep, (s + 1) * rstep
                    nc.vector.tensor_tensor(
                        out=t1v[:, r0:r1],
                        in0=sv[:, r0:r1, :, 0:4],
                        in1=sv[:, r0:r1, :, 4:8],
                        op=OP.add,
                    )
                t2 = red_pool.tile([128, 2304], BF, tag="t2")
                t2v = t2[:].rearrange("p (h x w) -> p h x w", h=48, x=24, w=2)
                nc.vector.tensor_tensor(
                    out=t2v, in0=t1v[:, :, :, 0:2], in1=t1v[:, :, :, 2:4], op=OP.add
                )
                # h-direction 8:1 before the final w-pair: (6y, 8r, 48xw)
                t2r = t2[:].rearrange("p (y r s) -> p y r s", y=6, r=8, s=48)
                t3 = red_pool.tile([128, 1152], BF, tag="t3")
                t3v = t3[:].rearrange("p (y r s) -> p y r s", y=6, r=4, s=48)
                nc.vector.tensor_tensor(
                    out=t3v, in0=t2r[:, :, 0:4, :], in1=t2r[:, :, 4:8, :], op=OP.add
                )
                t4 = red_pool.tile([128, 576], BF, tag="t4")
                t4v = t4[:].rearrange("p (y r s) -> p y r s", y=6, r=2, s=48)
                nc.vector.tensor_tensor(
                    out=t4v, in0=t3v[:, :, 0:2, :], in1=t3v[:, :, 2:4, :], op=OP.add
                )
                t5 = red_pool.tile([128, 288], BF, tag="t5")
                t5v = t5[:].rearrange("p (y r s) -> p y r s", y=6, r=1, s=48)
                nc.vector.tensor_tensor(
                    out=t5v, in0=t4v[:, :, 0:1, :], in1=t4v[:, :, 1:2, :], op=OP.add
                )
                # final w-pair written straight into the x-padded 26-wide row
                # layout: pooled [128=(c,q), (6y, 26x)], cols 1..24 are data
                t5w = t5[:].rearrange("p (y x w) -> p y x w", y=6, x=24, w=2)
                pooled = red_pool.tile([128, 6 * 26], BF, tag="pooled")
                pv26 = pooled[:].rearrange("p (y x) -> p y x", y=6, x=26)
                pv0 = pv26[:, :, 1:25].rearrange("p y (x w) -> p y x w", w=1)
                nc.vector.tensor_tensor(
                    out=pv0, in0=t5w[:, :, :, 0:1], in1=t5w[:, :, :, 1:2], op=OP.add
                )
                # x reflect pads, lane-local (cols 0,25 <- cols 2,23).
                # Mid-stream on ACT (DVE is the pacing engine there); the
                # last group's on the then-idle DVE for minimum tail latency.
                if g == NG - 1:
                    nc.vector.tensor_copy(pv26[:, :, 0:1], pv26[:, :, 2:3])
                    nc.vector.tensor_copy(pv26[:, :, 25:26], pv26[:, :, 23:24])
                else:
                    nc.scalar.copy(out=pv26[:, :, 0:1], in_=pv26[:, :, 2:3])
                    nc.scalar.copy(out=pv26[:, :, 25:26], in_=pv26[:, :, 23:24])

                # ---- build xrep [96=(dy,c), 24y, 26x] on the TENSOR engine.
                # Selector matmuls (lhsT = column-permuted identity slice)
                # remap partitions (c,q) -> (dy,c) and place dy-shifted,
                # reflect-padded row windows — no DMA involved, so the conv
                # never queues behind the HBM slab stream.  Two PSUM tiles
                # hold element halves 0:312 / 312:624 of the 624-el rows.
                psA = psx_pool.tile([96, 312], F32, tag="xpsA")
                psB = psx_pool.tile([96, 312], F32, tag="xpsB")
                # (q, dy-block, tile, dest el0:el1, src el0:el1); dy block d
                # writes row r from source row r + d - 1 (reflect at edges)
                pieces = [
                    (0, 0, psA, 0, 26, 26, 52),      # d0 row0 <- reflect row1
                    (0, 0, psA, 26, 182, 0, 156),    # d0 rows1-6 <- q0 rows0-5
                    (0, 1, psA, 0, 156, 0, 156),     # d1 rows0-5
                    (0, 2, psA, 0, 130, 26, 156),    # d2 rows0-4 <- q0 rows1-5
                    (1, 0, psA, 182, 312, 0, 130),   # d0 rows7-11
                    (1, 0, psB, 0, 26, 130, 156),    # d0 row12
                    (1, 1, psA, 156, 312, 0, 156),   # d1 rows6-11
                    (1, 2, psA, 130, 286, 0, 156),   # d2 rows5-10
                    (2, 0, psB, 26, 182, 0, 156),    # d0 rows13-18
                    (2, 1, psB, 0, 156, 0, 156),     # d1 rows12-17
                    (2, 2, psA, 286, 312, 0, 26),    # d2 row11
                    (2, 2, psB, 0, 130, 26, 156),    # d2 rows12-16
                    (3, 0, psB, 182, 312, 0, 130),   # d0 rows19-23
                    (3, 1, psB, 156, 312, 0, 156),   # d1 rows18-23
                    (3, 2, psB, 130, 286, 0, 156),   # d2 rows17-22
                    (3, 2, psB, 286, 312, 104, 130), # d2 row23 <- reflect row22
                ]
                for q, dblk, psX, e0, e1, s0, s1 in pieces:
                    nc.tensor.matmul(
                        psX[dblk * 32 : (dblk + 1) * 32, e0:e1],
                        sel_sb[:, q * 32 : (q + 1) * 32],
                        pooled[:, s0:s1],
                        start=True,
                        stop=True,
                    )
                xrep = xrep_pool.tile([96, 24 * 26], BF, tag="xrep")
                # last group: DVE is idle by now and is faster PSUM->SBUF;
                # mid-stream: keep these off the busy DVE
                ceng = nc.vector.tensor_copy if g == NG - 1 else (
                    lambda out, in_: nc.scalar.copy(out=out, in_=in_)
                )
                ceng(xrep[:, 0:312], psA[:])
                ceng(xrep[:, 312:624], psB[:])
                xr3 = xrep[:].rearrange("p (y x) -> p y x", y=OUT, x=26)

                if DEBUG_TAPS:
                    nc.gpsimd.dma_start(
                        out=pooled_out.ap()[g * 32 : (g + 1) * 32],
                        in_=xr3[32:64, :, 1:25],
                    )

                # ---- conv: 2 halves x 3 dx matmuls, K=(dy,ic)=96 ------------
                for half in range(2):
                    pc = psc_pool.tile([32, 288], F32, tag="convps")
                    for dx in range(3):
                        nc.tensor.matmul(
                            pc[:],
                            wt_sb[:, (m * 3 + dx) * 32 : (m * 3 + dx + 1) * 32],
                            xr3[:, 12 * half : 12 * half + 12, dx : dx + 24],
                            start=(dx == 0),
                            stop=(dx == 2),
                        )
                    # LeakyReLU(0.2) on the scalar engine, PSUM -> SBUF bf16
                    nc.scalar.activation(
                        out=v_sb[
                            m * 32 : (m + 1) * 32,
                            t * PIX + half * 288 : t * PIX + (half + 1) * 288,
                        ],
                        in_=pc[:],
                        func=ACT.Prelu,
                        alpha=0.2,
                    )

                # ---- Gram chunks: t0's five chunks spread over groups
                # 3/4/5 keep the tensor engine from idling >3.4us
                # mid-stream (HAM re-throttle), so tail matmuls run warm.
                def gram_chunks(gt, c5s, last_group):
                    gp = gram_ps[gt]
                    for c5 in c5s:
                        sz = 128 if c5 < 4 else 64
                        vsl = v_sb[
                            :, gt * PIX + c5 * 128 : gt * PIX + c5 * 128 + sz
                        ]
                        pt = pst_pool.tile([128, 96], BF, tag="vtps")
                        nc.tensor.transpose(pt[:sz, :], vsl, id_sb[:])
                        vtt = vt_pool.tile([128, 96], BF, tag="vt")
                        if last_group:
                            nc.vector.tensor_copy(vtt[:sz, :], pt[:sz, :])
                        else:
                            nc.scalar.copy(out=vtt[:sz, :], in_=pt[:sz, :])
                        nc.tensor.matmul(
                            gp[:], vtt[:sz, :], vtt[:sz, :],
                            start=(c5 == 0), stop=(c5 == 4),
                        )
                    if c5s[-1] == 4:
                        nc.scalar.copy(
                            out=g_sb[:, gt * 96 : (gt + 1) * 96], in_=gp[:]
                        )

                if g == 3:
                    gram0 = psg_pool.tile([96, 96], F32, tag="gram0")
                    gram_ps[0] = gram0
                    gram_chunks(0, [0, 1], False)
                elif g == 4:
                    gram_chunks(0, [2, 3], False)
                elif g == 5:
                    gram_chunks(0, [4], False)
                    gram1 = psg_pool.tile([96, 96], F32, tag="gram1")
                    gram_ps[1] = gram1
                    gram_chunks(1, [0, 1, 2, 3, 4], True)
            # one store for both timesteps' Grams
            nc.sync.dma_start(
                out=g_out.ap().rearrange("t i k -> i t k"), in_=g_sb[:]
            )

    nc.finalize()
    return nc


def _get_nc():
    if "nc" not in _STATE:
        _STATE["nc"] = _build_nc()
    return _STATE["nc"]


def _prep_weights(W1, W2, W3):
    # wt[(dy,ic), (m,dx,oc)] = W_m[oc, ic, dy, dx] / 64   (pool-mean folded in)
    w = np.stack([np.asarray(Wi, np.float64) for Wi in (W1, W2, W3)])
    wt = w.transpose(3, 2, 0, 4, 1).reshape(96, 288) / 64.0
    return wt.astype(ml_dtypes.bfloat16)


def _host_loss(G):
    G = np.asarray(G, np.float64)  # [16, 96, 96]
    T = G.shape[0]
    I96 = np.eye(M)
    Me = I96[None] + ALPHA_E * G
    ld_e = 2.0 * np.log(
        np.diagonal(np.linalg.cholesky(Me), axis1=-2, axis2=-1)
    ).sum()
    blocks = np.stack(
        [G[:, 32 * c : 32 * (c + 1), 32 * c : 32 * (c + 1)] for c in range(3)]
    )  # [3, T, 32, 32]
    Mc = np.eye(32)[None, None] + ALPHA_C * blocks
    ld_c = 2.0 * np.log(
        np.diagonal(np.linalg.cholesky(Mc), axis1=-2, axis2=-1)
    ).sum()
    loss_expd = ld_e / (2.0 * T)
    loss_comp = (32.0 / M) * ld_c / (2.0 * T)
    return np.float32(loss_expd - loss_comp)


def run_device(inputs, **kw):
    """Run the bass kernel; returns (G [16,96,96], BassKernelResults)."""
    from concourse.bass_utils import run_bass_kernel_spmd

    nc = _get_nc()
    wt = _prep_weights(inputs["W1"], inputs["W2"], inputs["W3"])
    ident = np.eye(96, dtype=ml_dtypes.bfloat16)
    # sel[:, q*32+c] = e_{c*4+q}
    perm = np.arange(128).reshape(4, 32).T.reshape(-1) * 0
    perm = np.array([(i % 32) * 4 + (i // 32) for i in range(128)])
    selm = np.eye(128)[:, perm].astype(ml_dtypes.bfloat16)
    ms = np.asarray(inputs["ms_fea"], np.float32)
    pan = np.asarray(inputs["pan_fea"], np.float32)
    alf = np.asarray(inputs["all_fea"], np.float32)
    in_maps = []
    for i in range(NCORES):
        sl = slice(TPC * i, TPC * (i + 1))
        # x[t*3+m] = (ms,pan,alf)[m][t]
        xs = np.stack([ms[sl], pan[sl], alf[sl]], axis=1).reshape(
            NG, CCH, H, W
        )
        in_maps.append(
            {"x": xs.astype(ml_dtypes.float8_e4m3fn), "wt": wt, "ident": ident,
             "sel": selm}
        )
    res = run_bass_kernel_spmd(nc, in_maps, core_ids=list(range(NCORES)), **kw)
    G = np.concatenate([np.asarray(r["g_out"]) for r in res.results], axis=0)
    return G, res


def kernel(**inputs):
    G, _ = run_device(inputs)
    return _host_loss(G)


# revision 39
# speedup vs baseline: 1.0276x; 1.0047x over previous
"""MCR loss kernel for Trainium2 (8 NeuronCores).  ~60us/core HW exec.

Strategy:
  - Shard batch T=16 -> 2 timesteps per core (data parallel, no collectives).
  - Host converts inputs to fp8 e4m3 (loss rel err ~6e-4, gate is 2e-2):
    quarters HBM traffic vs f32.  SWDGE cast-DMAs (gpsimd ring) upconvert
    fp8->bf16 on the fly so the vector engine keeps its 2x packed mode.
  - Per core, 6 plane-groups (2 timesteps x 3 maps), each one contiguous
    slab into [128=(channel,quarter), 9216] bf16, all dispatched up front
    (split-slab DMAs: quarters for group 0, halves after, for pipelining).
  - 8x8 avg-pool (as sum; the 1/64 mean is folded into the conv weights)
    as a pairwise tensor_tensor tree on the vector engine (bf16 2x mode,
    ~5.5us/slab = the stream pace-setter), final level writing straight
    into the x-reflect-padded 26-wide row layout.
  - The conv input im2col ("xrep": 3 dy-shifted, reflect-padded copies on
    partitions (dy,c)) is built ON THE TENSOR ENGINE with 16 selector
    matmuls per group (lhsT = column-permuted identity).  DMA-based
    construction loses: small DMAs complete only after the whole in-flight
    slab stream at the SDMA level, serializing every conv behind the load.
  - 3x3 conv as 3 PE matmuls, K=(dy,ic)=96, bf16; LeakyReLU(0.2) natively
    on the scalar engine (Prelu w/ alpha — Lrelu's table slope is hardwired
    0.01 and ignores alpha).
  - Gram G_t = V_t V_t^T via PE transpose + bf16 matmul (f32 PSUM accum);
    t0's chunks are spread over groups 3-5 so the PE never idles past the
    HAM re-throttle window mid-stream.
  - Host: matrix determinant lemma
        logdet(I_576 + a V^T V) = logdet(I_96 + a V V^T)
    so only the [2,96,96] Grams leave the device; float64 Cholesky logdets
    finish the scalar loss.
"""

import numpy as np
import ml_dtypes

_STATE = {}

# -------- fixed problem geometry (hardcoded per harness contract) --------
B, CCH, H, W = 16, 32, 192, 192
NCORES = 8
TPC = B // NCORES          # timesteps per core = 2
NG = TPC * 3               # plane groups per core
OUT = 24                   # pooled spatial size
PIX = OUT * OUT            # 576
M = 96                     # feature rows (3 maps x 32 channels)
ALPHA_E = 6.0              # 576 / (96 * eps)
ALPHA_C = 18.0             # 576 / (32 * eps)

DEBUG_TAPS = False


def _build_nc():
    import concourse.bass as bass
    import concourse.tile as tile
    from concourse import bacc, mybir

    BF = mybir.dt.bfloat16
    F8 = mybir.dt.float8e4
    F32 = mybir.dt.float32
    ACT = mybir.ActivationFunctionType
    OP = mybir.AluOpType

    nc = bacc.Bacc(
        "TRN2", target_bir_lowering=False, debug=False, num_devices=NCORES
    )

    x = nc.declare_dram_parameter("x", [NG, CCH, H, W], F8, isOutput=False)
    wt = nc.declare_dram_parameter("wt", [96, 288], BF, isOutput=False)
    ident = nc.declare_dram_parameter("ident", [96, 96], BF, isOutput=False)
    # sel[:, q*32+c] = e_{c*4+q}: column-permuted identity; sel[:, q*32:...]
    # as matmul lhsT gathers partition (c,q) -> output row c
    sel = nc.declare_dram_parameter("sel", [128, 128], BF, isOutput=False)
    g_out = nc.declare_dram_parameter("g_out", [TPC, M, M], F32, isOutput=True)
    if DEBUG_TAPS:
        pooled_out = nc.declare_dram_parameter(
            "pooled_out", [NG * 32, PIX], F32, isOutput=True
        )
        v_out = nc.declare_dram_parameter("v_out", [96, TPC * PIX], F32, isOutput=True)

    with tile.TileContext(nc) as tc:
        with (
            tc.tile_pool(name="persist", bufs=1) as persist,
            tc.tile_pool(name="slab", bufs=1) as slab_pool,
            tc.tile_pool(name="red", bufs=3) as red_pool,
            tc.tile_pool(name="xrep", bufs=2) as xrep_pool,
            tc.tile_pool(name="vt", bufs=2) as vt_pool,
            tc.tile_pool(name="psc", bufs=2, space="PSUM") as psc_pool,
            tc.tile_pool(name="pst", bufs=2, space="PSUM") as pst_pool,
            tc.tile_pool(name="psg", bufs=1, space="PSUM") as psg_pool,
            tc.tile_pool(name="psx", bufs=1, space="PSUM") as psx_pool,
        ):
            wt_sb = persist.tile([96, 288], BF, tag="wt")
            nc.sync.dma_start(out=wt_sb[:], in_=wt.ap())
            id_sb = persist.tile([96, 96], BF, tag="id")
            nc.sync.dma_start(out=id_sb[:], in_=ident.ap())
            sel_sb = persist.tile([128, 128], BF, tag="sel")
            nc.sync.dma_start(out=sel_sb[:], in_=sel.ap())
            v_sb = persist.tile([96, TPC * PIX], BF, tag="v")
            g_sb = persist.tile([96, TPC * 96], F32, tag="g")

            # ---- prefetch all 6 slabs up front: each is one contiguous
            # 2.25MB DMA, partition=(c, quarter).  Dispatching them before
            # any small DMA keeps the 8 shared HWDGE completion-semaphore
            # lanes fresh — interleaving would chain slab loads behind tiny
            # sbuf-to-sbuf DMAs from two groups earlier.
            slabs = []
            for g in range(NG):
                slab = slab_pool.tile([128, 9216], BF, tag=f"slab{g}")
                # SWDGE cast-DMA: reads fp8 from HBM, writes bf16 to SBUF.
                # Split-slab DMAs for finer pipelining: slab 0 in quarters
                # (the very first L1 gates the whole pipeline), the rest in
                # halves.
                xg = x.ap()[g].rearrange("c (q h) w -> (c q) (h w)", q=4)
                nsplit = 4 if g == 0 else 2
                step = 9216 // nsplit
                for s in range(nsplit):
                    nc.gpsimd.dma_start(
                        out=slab[:, s * step : (s + 1) * step],
                        in_=xg[:, s * step : (s + 1) * step],
                    )
                slabs.append(slab)

            gram_ps = {}
            for g in range(NG):
                t, m = divmod(g, 3)
                slab = slabs[g]

                # ---- pooling: pairwise tensor_tensor tree (bf16 2x mode) ----
                # per partition: 48 rows x 192 cols = (48h, 24x, 8w).
                # L1 split into row-blocks matching the split-slab DMAs.
                sv = slab[:].rearrange("p (h x w) -> p h x w", h=48, x=24, w=8)
                t1 = red_pool.tile([128, 4608], BF, tag="t1")
                t1v = t1[:].rearrange("p (h x w) -> p h x w", h=48, x=24, w=4)
                nsplit = 4 if g == 0 else 2
                rstep = 48 // nsplit
                for s in range(nsplit):
                    r0, r1 = s * rstep, (s + 1) * rstep
                    nc.vector.tensor_tensor(
                        out=t1v[:, r0:r1],
                        in0=sv[:, r0:r1, :, 0:4],
                        in1=sv[:, r0:r1, :, 4:8],
                        op=OP.add,
                    )
                t2 = red_pool.tile([128, 2304], BF, tag="t2")
                t2v = t2[:].rearrange("p (h x w) -> p h x w", h=48, x=24, w=2)
                nc.vector.tensor_tensor(
                    out=t2v, in0=t1v[:, :, :, 0:2], in1=t1v[:, :, :, 2:4], op=OP.add
                )
                # h-direction 8:1 before the final w-pair: (6y, 8r, 48xw)
                t2r = t2[:].rearrange("p (y r s) -> p y r s", y=6, r=8, s=48)
                t3 = red_pool.tile([128, 1152], BF, tag="t3")
                t3v = t3[:].rearrange("p (y r s) -> p y r s", y=6, r=4, s=48)
                nc.vector.tensor_tensor(
                    out=t3v, in0=t2r[:, :, 0:4, :], in1=t2r[:, :, 4:8, :], op=OP.add
                )
                t4 = red_pool.tile([128, 576], BF, tag="t4")
                t4v = t4[:].rearrange("p (y r s) -> p y r s", y=6, r=2, s=48)
                nc.vector.tensor_tensor(
                    out=t4v, in0=t3v[:, :, 0:2, :], in1=t3v[:, :, 2:4, :], op=OP.add
                )
                t5 = red_pool.tile([128, 288], BF, tag="t5")
                t5v = t5[:].rearrange("p (y r s) -> p y r s", y=6, r=1, s=48)
                nc.vector.tensor_tensor(
                    out=t5v, in0=t4v[:, :, 0:1, :], in1=t4v[:, :, 1:2, :], op=OP.add
                )
                # final w-pair written straight into the x-padded 26-wide row
                # layout: pooled [128=(c,q), (6y, 26x)], cols 1..24 are data
                t5w = t5[:].rearrange("p (y x w) -> p y x w", y=6, x=24, w=2)
                pooled = red_pool.tile([128, 6 * 26], BF, tag="pooled")
                pv26 = pooled[:].rearrange("p (y x) -> p y x", y=6, x=26)
                pv0 = pv26[:, :, 1:25].rearrange("p y (x w) -> p y x w", w=1)
                nc.vector.tensor_tensor(
                    out=pv0, in0=t5w[:, :, :, 0:1], in1=t5w[:, :, :, 1:2], op=OP.add
                )
                # x reflect pads, lane-local (cols 0,25 <- cols 2,23).
                # Mid-stream on ACT (DVE is the pacing engine there); the
                # last group's on the then-idle DVE for minimum tail latency.
                if g == NG - 1:
                    nc.vector.tensor_copy(pv26[:, :, 0:1], pv26[:, :, 2:3])
                    nc.vector.tensor_copy(pv26[:, :, 25:26], pv26[:, :, 23:24])
                else:
                    nc.scalar.copy(out=pv26[:, :, 0:1], in_=pv26[:, :, 2:3])
                    nc.scalar.copy(out=pv26[:, :, 25:26], in_=pv26[:, :, 23:24])

                # ---- build xrep [96=(dy,c), 24y, 26x] on the TENSOR engine.
                # Selector matmuls (lhsT = column-permuted identity slice)
                # remap partitions (c,q) -> (dy,c) and place dy-shifted,
                # reflect-padded row windows — no DMA involved, so the conv
                # never queues behind the HBM slab stream.  Two PSUM tiles
                # hold element halves 0:312 / 312:624 of the 624-el rows.
                psA = psx_pool.tile([96, 312], F32, tag="xpsA")
                psB = psx_pool.tile([96, 312], F32, tag="xpsB")
                # (q, dy-block, tile, dest el0:el1, src el0:el1); dy block d
                # writes row r from source row r + d - 1 (reflect at edges)
                pieces = [
                    (0, 0, psA, 0, 26, 26, 52),      # d0 row0 <- reflect row1
                    (0, 0, psA, 26, 182, 0, 156),    # d0 rows1-6 <- q0 rows0-5
                    (0, 1, psA, 0, 156, 0, 156),     # d1 rows0-5
                    (0, 2, psA, 0, 130, 26, 156),    # d2 rows0-4 <- q0 rows1-5
                    (1, 0, psA, 182, 312, 0, 130),   # d0 rows7-11
                    (1, 0, psB, 0, 26, 130, 156),    # d0 row12
                    (1, 1, psA, 156, 312, 0, 156),   # d1 rows6-11
                    (1, 2, psA, 130, 286, 0, 156),   # d2 rows5-10
                    (2, 0, psB, 26, 182, 0, 156),    # d0 rows13-18
                    (2, 1, psB, 0, 156, 0, 156),     # d1 rows12-17
                    (2, 2, psA, 286, 312, 0, 26),    # d2 row11
                    (2, 2, psB, 0, 130, 26, 156),    # d2 rows12-16
                    (3, 0, psB, 182, 312, 0, 130),   # d0 rows19-23
                    (3, 1, psB, 156, 312, 0, 156),   # d1 rows18-23
                    (3, 2, psB, 130, 286, 0, 156),   # d2 rows17-22
                    (3, 2, psB, 286, 312, 104, 130), # d2 row23 <- reflect row22
                ]
                for q, dblk, psX, e0, e1, s0, s1 in pieces:
                    nc.tensor.matmul(
                        psX[dblk * 32 : (dblk + 1) * 32, e0:e1],
                        sel_sb[:, q * 32 : (q + 1) * 32],
                        pooled[:, s0:s1],
                        start=True,
                        stop=True,
                    )
                xrep = xrep_pool.tile([96, 24 * 26], BF, tag="xrep")
                # last group: DVE is idle by now and is faster PSUM->SBUF;
                # mid-stream: keep these off the busy DVE
                ceng = nc.vector.tensor_copy if g == NG - 1 else (
                    lambda out, in_: nc.scalar.copy(out=out, in_=in_)
                )
                ceng(xrep[:, 0:312], psA[:])
                ceng(xrep[:, 312:624], psB[:])
                xr3 = xrep[:].rearrange("p (y x) -> p y x", y=OUT, x=26)

                if DEBUG_TAPS:
                    nc.gpsimd.dma_start(
                        out=pooled_out.ap()[g * 32 : (g + 1) * 32],
                        in_=xr3[32:64, :, 1:25],
                    )

                # ---- conv: 2 halves x 3 dx matmuls, K=(dy,ic)=96 ------------
                for half in range(2):
                    pc = psc_pool.tile([32, 288], F32, tag="convps")
                    for dx in range(3):
                        nc.tensor.matmul(
                            pc[:],
                            wt_sb[:, (m * 3 + dx) * 32 : (m * 3 + dx + 1) * 32],
                            xr3[:, 12 * half : 12 * half + 12, dx : dx + 24],
                            start=(dx == 0),
                            stop=(dx == 2),
                        )
                    # LeakyReLU(0.2) on the scalar engine, PSUM -> SBUF bf16
                    nc.scalar.activation(
                        out=v_sb[
                            m * 32 : (m + 1) * 32,
                            t * PIX + half * 288 : t * PIX + (half + 1) * 288,
                        ],
                        in_=pc[:],
                        func=ACT.Prelu,
                        alpha=0.2,
                    )

                # ---- Gram chunks: t0's five chunks spread over groups
                # 3/4/5 keep the tensor engine from idling >3.4us
                # mid-stream (HAM re-throttle), so tail matmuls run warm.
                def gram_chunks(gt, c5s, last_group):
                    gp = gram_ps[gt]
                    for c5 in c5s:
                        sz = 128 if c5 < 4 else 64
                        vsl = v_sb[
                            :, gt * PIX + c5 * 128 : gt * PIX + c5 * 128 + sz
                        ]
                        pt = pst_pool.tile([128, 96], BF, tag="vtps")
                        nc.tensor.transpose(pt[:sz, :], vsl, id_sb[:])
                        vtt = vt_pool.tile([128, 96], BF, tag="vt")
                        if last_group:
                            nc.vector.tensor_copy(vtt[:sz, :], pt[:sz, :])
                        else:
                            nc.scalar.copy(out=vtt[:sz, :], in_=pt[:sz, :])
                        nc.tensor.matmul(
                            gp[:], vtt[:sz, :], vtt[:sz, :],
                            start=(c5 == 0), stop=(c5 == 4),
                        )
                    if c5s[-1] == 4:
                        nc.scalar.copy(
                            out=g_sb[:, gt * 96 : (gt + 1) * 96], in_=gp[:]
                        )

                if g == 3:
                    gram0 = psg_pool.tile([96, 96], F32, tag="gram0")
                    gram_ps[0] = gram0
                    gram_chunks(0, [0, 1], False)
                elif g == 4:
                    gram_chunks(0, [2, 3], False)
                elif g == 5:
                    gram_chunks(0, [4], False)
                    gram1 = psg_pool.tile([96, 96], F32, tag="gram1")
                    gram_ps[1] = gram1
                    gram_chunks(1, [0, 1, 2, 3, 4], True)
            # one store for both timesteps' Grams
            nc.sync.dma_start(
                out=g_out.ap().rearrange("t i k -> i t k"), in_=g_sb[:]
            )

    nc.finalize()
    return nc


def _get_nc():
    if "nc" not in _STATE:
        _STATE["nc"] = _build_nc()
    return _STATE["nc"]


def _prep_weights(W1, W2, W3):
    # wt[(dy,ic), (m,dx,oc)] = W_m[oc, ic, dy, dx] / 64   (pool-mean folded in)
    w = np.stack([np.asarray(Wi, np.float64) for Wi in (W1, W2, W3)])
    wt = w.transpose(3, 2, 0, 4, 1).reshape(96, 288) / 64.0
    return wt.astype(ml_dtypes.bfloat16)


def _host_loss(G):
    G = np.asarray(G, np.float64)  # [16, 96, 96]
    T = G.shape[0]
    I96 = np.eye(M)
    Me = I96[None] + ALPHA_E * G
    ld_e = 2.0 * np.log(
        np.diagonal(np.linalg.cholesky(Me), axis1=-2, axis2=-1)
    ).sum()
    blocks = np.stack(
        [G[:, 32 * c : 32 * (c + 1), 32 * c : 32 * (c + 1)] for c in range(3)]
    )  # [3, T, 32, 32]
    Mc = np.eye(32)[None, None] + ALPHA_C * blocks
    ld_c = 2.0 * np.log(
        np.diagonal(np.linalg.cholesky(Mc), axis1=-2, axis2=-1)
    ).sum()
    loss_expd = ld_e / (2.0 * T)
    loss_comp = (32.0 / M) * ld_c / (2.0 * T)
    return np.float32(loss_expd - loss_comp)


def run_device(inputs, **kw):
    """Run the bass kernel; returns (G [16,96,96], BassKernelResults)."""
    from concourse.bass_utils import run_bass_kernel_spmd

    nc = _get_nc()
    wt = _prep_weights(inputs["W1"], inputs["W2"], inputs["W3"])
    ident = np.eye(96, dtype=ml_dtypes.bfloat16)
    # sel[:, q*32+c] = e_{c*4+q}
    perm = np.arange(128).reshape(4, 32).T.reshape(-1) * 0
    perm = np.array([(i % 32) * 4 + (i // 32) for i in range(128)])
    selm = np.eye(128)[:, perm].astype(ml_dtypes.bfloat16)
    ms = np.asarray(inputs["ms_fea"], np.float32)
    pan = np.asarray(inputs["pan_fea"], np.float32)
    alf = np.asarray(inputs["all_fea"], np.float32)
    in_maps = []
    for i in range(NCORES):
        sl = slice(TPC * i, TPC * (i + 1))
        # x[t*3+m] = (ms,pan,alf)[m][t]
        xs = np.stack([ms[sl], pan[sl], alf[sl]], axis=1).reshape(
            NG, CCH, H, W
        )
        in_maps.append(
            {"x": xs.astype(ml_dtypes.float8_e4m3fn), "wt": wt, "ident": ident,
             "sel": selm}
        )
    res = run_bass_kernel_spmd(nc, in_maps, core_ids=list(range(NCORES)), **kw)
    G = np.concatenate([np.asarray(r["g_out"]) for r in res.results], axis=0)
    return G, res


def kernel(**inputs):
    G, _ = run_device(inputs)
    return _host_loss(G)
